# revision 1
# baseline (speedup 1.0000x reference)
"""PolyCntSketch (TensorSketch, degree 3) Trainium2 kernel.

Math: for each degree d, CountSketch_d = X @ S_d (S_d one-hot signed), then
out = irfft(prod_d rfft(CountSketch_d)).

Device strategy (pure data parallelism over batch, 8 cores):
  - Host feeds X transposed ([F, B_core]), features packed into 128-row chunks
    where each chunk holds whole (block_d0, block_d1, block_d2)-classes
    (block = idx_d // 128), so each chunk touches few 128-bucket blocks per
    degree -> few segment matmuls.
  - Stage 1: count sketch via per-(chunk, degree, block) matmuls
    (lhsT = signed one-hot Z [fill, 128]) accumulated in PSUM -> sketch [512, B].
  - Stage 2: rfft as DFT matmul (K = 512 buckets) -> Re/Im [256+Nyquist, B].
  - Stage 3: complex product across the 3 degrees on VectorE; the Nyquist bin
    (pure real) rides in the identically-zero Im(0) slot.
  - Stage 4: irfft as matmul -> out^T [512, B].
All matmuls run in float32r (fp32 rounded to 11-bit mantissa; full PE rate at
N >= 256). Host pre-rounds inputs so DMA can feed fp32r tiles directly.
"""
import sys

for _p in ("/opt/trn_rl_repo",):
    if _p not in sys.path:
        sys.path.append(_p)

import numpy as np

from concourse import bacc, mybir, tile
from concourse import bass_utils

F32R = mybir.dt.float32r
F32 = mybir.dt.float32

B, F, NCOMP, DEG = 8192, 4096, 512, 3
NCORES = 8
B_CORE = B // NCORES
BT = 256                     # batch columns per tile
CHUNK = 128
NBLK = NCOMP // 128          # 4 bucket blocks


def round_fp32r(x):
    b = np.ascontiguousarray(x, np.float32).view(np.uint32)
    t = b + 0x7FF + ((b >> 12) & 1)
    return (t & 0xFFFFF000).astype(np.uint32).view(np.float32)


def build_plan(index_hash, bit_hash):
    """FFD-pack whole (g0,g1,g2)-classes into 128-row chunks.

    Returns:
      order [F]: feature order for the transposed X upload
      chunks: list of (start, fill) row ranges into the ordered X
      plan[d][g]: list of (chunk_idx, zslot) in emission order ((d,g)-major)
      zm_t [128, nmm, 128]: stacked Z matrices, partition-major
    """
    idx = np.asarray(index_hash)
    sgn = (np.asarray(bit_hash) * 2 - 1).astype(np.float32)
    blocks = idx >> 7
    key = blocks[0] * 16 + blocks[1] * 4 + blocks[2]
    order_all = np.argsort(key, kind="stable")
    kvals = key[order_all]

    from collections import defaultdict
    sgroups = defaultdict(list)
    for kv in np.unique(kvals):
        f = order_all[kvals == kv]
        kv = int(kv)
        while len(f) > CHUNK:
            sgroups[kv >> 2].append((kv, f[:CHUNK]))
            f = f[CHUNK:]
        if len(f):
            sgroups[kv >> 2].append((kv, f))

    bins = []
    for sg in sorted(sgroups):
        sbins = []
        for kv, f in sorted(sgroups[sg], key=lambda x: -len(x[1])):
            for b in sbins:
                if sum(len(x[1]) for x in b) + len(f) <= CHUNK:
                    b.append((kv, f))
                    break
            else:
                sbins.append([(kv, f)])
        bins.extend(sbins)
    # merge small bins globally (saves chunks at the cost of 1-2 extra mms)
    bins.sort(key=lambda b: sum(len(x[1]) for x in b))
    merged = []
    while bins:
        b = bins.pop(0)
        size = sum(len(x[1]) for x in b)
        for i in range(len(bins) - 1, -1, -1):
            if sum(len(x[1]) for x in bins[i]) + size <= CHUNK:
                bins[i].extend(b)
                break
        else:
            merged.append(b)
    merged.sort(key=lambda b: min(x[0] for x in b))

    order = []
    chunks = []
    for b in merged:
        start = len(order)
        for kv, f in b:
            order.extend(f.tolist())
        chunks.append((start, len(order) - start))
    order = np.array(order)
    assert len(order) == F and len(np.unique(order)) == F

    items = [[[] for _ in range(NBLK)] for _ in range(DEG)]
    for ci, (start, fill) in enumerate(chunks):
        feats = order[start:start + fill]
        for d in range(DEG):
            for g in np.unique(blocks[d, feats]):
                g = int(g)
                rows = np.nonzero(blocks[d, feats] == g)[0]
                Z = np.zeros((CHUNK, 128), np.float32)
                Z[rows, idx[d, feats[rows]] - 128 * g] = sgn[d, feats[rows]]
                items[d][g].append((ci, Z))
    for d in range(DEG):
        for g in range(NBLK):
            if not items[d][g]:
                items[d][g].append((0, np.zeros((CHUNK, 128), np.float32)))

    zmats = []
    plan = [[[] for _ in range(NBLK)] for _ in range(DEG)]
    for d in range(DEG):
        for g in range(NBLK):
            for (ci, Z) in sorted(items[d][g], key=lambda x: x[0]):
                plan[d][g].append((ci, len(zmats)))
                zmats.append(Z)
    zm = np.stack(zmats)                                # [nmm, 128, 128]
    zm_t = np.ascontiguousarray(zm.transpose(1, 0, 2))  # [128, nmm, 128]
    return order, chunks, plan, zm_t


def build_dft_tables():
    n = np.arange(NCOMP)[:, None]
    k = np.arange(257)[None, :]
    ang = 2 * np.pi * n * k / NCOMP
    # stage-2 lhsT [512, 514]: cols 0..256 Re coeffs, cols 257..513 Im coeffs
    dft = np.concatenate([np.cos(ang), -np.sin(ang)], axis=1).astype(np.float32)
    dft_t = np.ascontiguousarray(
        dft.reshape(4, 128, 514).transpose(1, 0, 2))    # [128, 4, 514]

    kk = np.arange(257)[:, None]
    nn = np.arange(NCOMP)[None, :]
    ang2 = 2 * np.pi * kk * nn / NCOMP
    ck = np.full((257, 1), 2.0, np.float32); ck[0] = 1.0
    dk = np.full((257, 1), 2.0, np.float32); dk[0] = 0.0
    ire = (ck * np.cos(ang2) / NCOMP).astype(np.float32)   # [257, 512]
    iim = (-dk * np.sin(ang2) / NCOMP).astype(np.float32)
    ico = np.zeros((4, 128, NCOMP), np.float32)
    ico[0] = ire[0:128]
    ico[1] = ire[128:256]
    ico[2] = iim[0:128]
    ico[3] = iim[128:256]
    # Nyquist rides in the identically-zero Im(0) slot: its product lands in
    # prod[Im-block-A, row 0], and the matching irfft column is (1/N)(-1)^n.
    ico[2, 0] = np.cos(np.pi * np.arange(NCOMP)).astype(np.float32) / NCOMP
    ico_t = np.ascontiguousarray(ico.transpose(1, 0, 2))   # [128, 4, 512]
    return dft_t, ico_t


def build_program(plan, chunks, nmm, b_core=B_CORE):
    nbt = b_core // BT
    nch = len(chunks)
    ngrp = (nch + 7) // 8
    fills = [f for (_, f) in chunks]
    # (d, g) -> [lo, hi) slice into the z stack
    zoff = {}
    pos = 0
    for d in range(DEG):
        for g in range(NBLK):
            zoff[(d, g)] = (pos, pos + len(plan[d][g]))
            pos += len(plan[d][g])
    assert pos == nmm

    nc = bacc.Bacc("TRN2", target_bir_lowering=False, debug=False)
    xp = nc.dram_tensor("xp", [len(chunks) * 128, b_core], F32R,
                        kind="ExternalInput").ap()
    zm = nc.dram_tensor("zm", [128, nmm, 128], F32R, kind="ExternalInput").ap()
    dft = nc.dram_tensor("dft", [128, 4, 514], F32R, kind="ExternalInput").ap()
    ico = nc.dram_tensor("ico", [128, 4, 512], F32R, kind="ExternalInput").ap()
    ot = nc.dram_tensor("ot", [NCOMP, b_core], F32, kind="ExternalOutput").ap()

    with tile.TileContext(nc) as tc:
        with (
            tc.tile_pool(name="pz", bufs=1) as pz,
            tc.tile_pool(name="pc", bufs=1) as pc,
            tc.tile_pool(name="px", bufs=2) as px,
            tc.tile_pool(name="psk", bufs=1) as psk,
            tc.tile_pool(name="pprod", bufs=2) as pprod,
            tc.tile_pool(name="ptmp", bufs=1) as ptmp,
            tc.tile_pool(name="pout", bufs=4) as pout,
            tc.tile_pool(name="ps_sk", bufs=2, space="PSUM") as ps_sk,
            tc.tile_pool(name="ps_fr", bufs=3, space="PSUM") as ps_fr,
            tc.tile_pool(name="ps_out", bufs=3, space="PSUM") as ps_out,
        ):
            zts = {}
            consts = {}
            prods = {}

            def emit_stage4(tt):
                prod = prods.pop(tt)
                icot = consts["icot"]
                for m in range(4):
                    po = ps_out.tile([128, BT], F32, tag="po")
                    for q in range(4):
                        nc.tensor.matmul(
                            po[:], icot[:, q, 128 * m:128 * (m + 1)],
                            prod[:, q, :],
                            start=(q == 0), stop=(q == 3))
                    ob = pout.tile([128, BT], F32, tag="ob")
                    nc.vector.tensor_copy(ob[:], po[:])
                    nc.scalar.dma_start(
                        ot[128 * m:128 * (m + 1), BT * tt:BT * (tt + 1)], ob[:])

            def load_z(d, g):
                lo, hi = zoff[(d, g)]
                zt = pz.tile([128, hi - lo, 128], F32R, tag=f"z{d}{g}")
                nc.sync.dma_start(zt[:], zm[:, lo:hi, :])
                zts[(d, g)] = zt

            def load_xg(xgs, j, t):
                if j in xgs:
                    return
                w = min(8, nch - 8 * j)
                xt = px.tile([128, w, BT], F32R, tag=f"xg{j}")
                src = xp[1024 * j:1024 * j + 128 * w, BT * t:BT * (t + 1)]
                nc.sync.dma_start(xt[:], src.rearrange("(c p) n -> p c n", p=128))
                xgs[j] = xt

            for t in range(nbt):
                xgs = {}
                if t == 0:
                    # startup-latency-ordered loads: the (0, g) Z pieces and
                    # the X groups they touch arrive first so PE starts ~ASAP
                    for g in range(NBLK):
                        load_z(0, g)
                        for (ci, _) in plan[0][g]:
                            load_xg(xgs, ci // 8, t)
                    for j in range(ngrp):
                        load_xg(xgs, j, t)
                    dftt = pc.tile([128, 4, 514], F32R, tag="dftt")
                    nc.sync.dma_start(dftt[:], dft[:])
                    icot = pc.tile([128, 4, 512], F32R, tag="icot")
                    nc.sync.dma_start(icot[:], ico[:])
                    consts["dftt"] = dftt
                    consts["icot"] = icot
                    for d in (1, 2):
                        for g in range(NBLK):
                            load_z(d, g)
                else:
                    for j in range(ngrp):
                        load_xg(xgs, j, t)
                dftt = consts["dftt"]
                icot = consts["icot"]

                prod = pprod.tile([128, 4, BT], F32R, tag="prod")

                # ---- stage 1 for ALL degrees first: keeps the in-order PE
                # queue saturated with independent matmuls while ACT/DVE chew
                # on copies, and lets stage 2 find its SBUF inputs ready.
                sksd = []
                ssums = []
                for d in range(DEG):
                    sks = []
                    for g in range(NBLK):
                        pssk = ps_sk.tile([128, BT], F32, tag="psk")
                        items = plan[d][g]
                        zt = zts[(d, g)]
                        for i, (ci, zi) in enumerate(items):
                            fill = fills[ci]
                            nc.tensor.matmul(
                                pssk[:],
                                zt[0:fill, i, :],
                                xgs[ci // 8][0:fill, ci % 8, :],
                                start=(i == 0),
                                stop=(i == len(items) - 1),
                            )
                        sk = psk.tile([128, BT], F32R, tag=f"sk{d}{g}")
                        nc.scalar.copy(sk[:], pssk[:])
                        sks.append(sk)
                    sksd.append(sks)
                    # alternating-sign sum feeding the Nyquist bin:
                    # Re(256) = sum_p (-1)^p (sk0+sk1+sk2+sk3)[p]
                    s01 = ptmp.tile([128, BT], F32, tag="t1")
                    s23 = ptmp.tile([128, BT], F32, tag="t2")
                    ssum = ptmp.tile([128, BT], F32R, tag=f"ss{d}")
                    nc.vector.tensor_add(s01[:], sks[0][:].bitcast(F32),
                                         sks[1][:].bitcast(F32))
                    nc.vector.tensor_add(s23[:], sks[2][:].bitcast(F32),
                                         sks[3][:].bitcast(F32))
                    nc.vector.tensor_add(ssum[:], s01[:], s23[:])
                    ssums.append(ssum)

                # ---- stage 4 of the PREVIOUS btile: its product is long done
                if t > 0:
                    emit_stage4(t - 1)

                # ---- stage 2 + 3 per degree
                for d in range(DEG):
                    sks = sksd[d]
                    for pi, (mre, mim) in enumerate(((0, 257), (128, 385))):
                        frre = ps_fr.tile([128, BT], F32, tag="fr")
                        frim = ps_fr.tile([128, BT], F32, tag="fr")
                        for q in range(4):
                            nc.tensor.matmul(
                                frre[:], dftt[:, q, mre:mre + 128], sks[q][:],
                                start=(q == 0), stop=(q == 3))
                        for q in range(4):
                            nc.tensor.matmul(
                                frim[:], dftt[:, q, mim:mim + 128], sks[q][:],
                                start=(q == 0), stop=(q == 3))
                        if pi == 0:
                            # Nyquist row via the summed sketch (one matmul)
                            frt = ps_out.tile([1, BT], F32, tag="po")
                            nc.tensor.matmul(
                                frt[:], dftt[:, 0, 256:257], ssums[d][:],
                                start=True, stop=True)
                            pret = prod[0:1, 2, :]
                            tT = None
                            if d > 0:
                                tT = ptmp.tile([1, BT], F32R, tag="tT")
                                nc.vector.tensor_mul(
                                    tT[:], pret.bitcast(F32), frt[:])
                        pre = prod[:, pi, :]
                        pim = prod[:, 2 + pi, :]
                        if d == 0:
                            nc.vector.tensor_copy(pre, frre[:])
                            nc.vector.tensor_copy(pim, frim[:])
                        else:
                            pre32 = pre.bitcast(F32)
                            pim32 = pim.bitcast(F32)
                            t1 = ptmp.tile([128, BT], F32, tag="t1")
                            t2 = ptmp.tile([128, BT], F32, tag="t2")
                            t3 = ptmp.tile([128, BT], F32, tag="t3")
                            t4 = ptmp.tile([128, BT], F32, tag="t4")
                            nc.vector.tensor_mul(t1[:], pre32, frre[:])
                            nc.vector.tensor_mul(t2[:], pim32, frim[:])
                            nc.vector.tensor_mul(t3[:], pre32, frim[:])
                            nc.vector.tensor_mul(t4[:], pim32, frre[:])
                            nc.vector.tensor_sub(pre, t1[:], t2[:])
                            nc.vector.tensor_add(pim, t3[:], t4[:])
                    if tT is not None:
                        nc.vector.tensor_copy(pret, tT[:])
                    else:
                        nc.vector.tensor_copy(pret, frt[:])
                prods[t] = prod
            emit_stage4(nbt - 1)

    nc.compile()
    return nc


def prepare_inputs(X, index_hash, bit_hash):
    order, chunks, plan, zm_t = build_plan(index_hash, bit_hash)
    dft_t, ico_t = build_dft_tables()
    nmm = zm_t.shape[1]
    # padded layout: chunk c occupies rows [128c, 128c+fill), rest zero
    Xt = round_fp32r(np.asarray(X, np.float32).T[order])
    Xp = np.zeros((len(chunks) * 128, Xt.shape[1]), np.float32)
    for c, (start, fill) in enumerate(chunks):
        Xp[128 * c:128 * c + fill] = Xt[start:start + fill]
    shared = {
        "zm": zm_t,                      # +-1/0: exact in fp32r already
        "dft": round_fp32r(dft_t),
        "ico": round_fp32r(ico_t),
    }
    return plan, chunks, nmm, Xp, shared


def kernel(X, index_hash, bit_hash):
    plan, chunks, nmm, Xp, shared = prepare_inputs(X, index_hash, bit_hash)
    nc = build_program(plan, chunks, nmm)
    in_maps = [
        {"xp": np.ascontiguousarray(Xp[:, c * B_CORE:(c + 1) * B_CORE]), **shared}
        for c in range(NCORES)
    ]
    res = bass_utils.run_bass_kernel_spmd(
        nc, in_maps, core_ids=list(range(NCORES)))
    out = np.empty((B, NCOMP), np.float32)
    for c in range(NCORES):
        out[c * B_CORE:(c + 1) * B_CORE] = res.results[c]["ot"].T
    return out



# revision 6
# speedup vs baseline: 1.1721x; 1.1721x over previous
"""PolyCntSketch (TensorSketch, degree 3) Trainium2 kernel — v2 (fp16).

Math: for each degree d, CountSketch_d = X @ S_d (S_d one-hot signed), then
out = irfft(prod_d rfft(CountSketch_d)).

Device strategy (pure data parallelism over batch, 8 cores):
  - Host feeds X transposed ([F, B_core]) in fp16, features packed into
    128-row chunks holding whole (block_d0, block_d1, block_d2)-classes
    (block = idx_d // 128), so each chunk touches few 128-bucket blocks per
    degree -> few segment matmuls.
  - Stage 1: count sketch via per-(chunk, degree, block) fp16 matmuls
    (lhsT = signed one-hot Z) accumulated in PSUM -> sketch [512, B].
  - Stage 2: rfft as fp16 DFT matmul, scaled by 1/16 to keep fp16 range.
    The Nyquist bin's coefficients ride in the identically-zero Im(0)
    weight column, so no extra sum/matmul is needed.
  - Stage 3: complex product across the 3 degrees on DVE in pure-SBUF fp16
    (2x mode); row 0 (DC & Nyquist, both real) fixed up with tiny ops.
  - Stage 4: irfft as fp16 matmul (table scaled by 16^3) -> out^T fp32.
Two batch tiles of 512 columns (one PSUM bank per matmul output).
"""
import sys

for _p in ("/opt/trn_rl_repo",):
    if _p not in sys.path:
        sys.path.append(_p)

import numpy as np

from concourse import bacc, mybir, tile
from concourse import bass_utils

F16 = mybir.dt.float16
F32 = mybir.dt.float32

B, F, NCOMP, DEG = 8192, 4096, 512, 3
NCORES = 8
B_CORE = B // NCORES
BT = 512                     # batch columns per tile
NBT = B_CORE // BT
CHUNK = 128
NBLK = NCOMP // 128          # 4 bucket blocks
GRP = 4                      # chunks per X-load group
S2SCALE = 1.0 / 16.0         # fp16 range scaling for the DFT stage


def build_plan(index_hash, bit_hash):
    """FFD-pack whole (g0,g1,g2)-classes into 128-row chunks.

    Returns:
      order [F]: feature order for the transposed X upload
      chunks: list of (start, fill) row ranges into the ordered X
      plan[d][g]: list of (chunk_idx, zslot) in emission order ((d,g)-major)
      zm_t [128, npair, 256]: stacked Z matrices, two per 256-col row
    """
    idx = np.asarray(index_hash)
    sgn = (np.asarray(bit_hash) * 2 - 1).astype(np.float32)
    blocks = idx >> 7
    key = blocks[0] * 16 + blocks[1] * 4 + blocks[2]
    order_all = np.argsort(key, kind="stable")
    kvals = key[order_all]

    from collections import defaultdict
    sgroups = defaultdict(list)
    for kv in np.unique(kvals):
        f = order_all[kvals == kv]
        kv = int(kv)
        while len(f) > CHUNK:
            sgroups[kv >> 2].append((kv, f[:CHUNK]))
            f = f[CHUNK:]
        if len(f):
            sgroups[kv >> 2].append((kv, f))

    bins = []
    for sg in sorted(sgroups):
        sbins = []
        for kv, f in sorted(sgroups[sg], key=lambda x: -len(x[1])):
            for b in sbins:
                if sum(len(x[1]) for x in b) + len(f) <= CHUNK:
                    b.append((kv, f))
                    break
            else:
                sbins.append([(kv, f)])
        bins.extend(sbins)
    # merge small bins globally (saves chunks at the cost of 1-2 extra mms)
    bins.sort(key=lambda b: sum(len(x[1]) for x in b))
    merged = []
    while bins:
        b = bins.pop(0)
        size = sum(len(x[1]) for x in b)
        for i in range(len(bins) - 1, -1, -1):
            if sum(len(x[1]) for x in bins[i]) + size <= CHUNK:
                bins[i].extend(b)
                break
        else:
            merged.append(b)
    merged.sort(key=lambda b: min(x[0] for x in b))

    order = []
    chunks = []
    for b in merged:
        start = len(order)
        for kv, f in b:
            order.extend(f.tolist())
        chunks.append((start, len(order) - start))
    order = np.array(order)
    assert len(order) == F and len(np.unique(order)) == F

    items = [[[] for _ in range(NBLK)] for _ in range(DEG)]
    for ci, (start, fill) in enumerate(chunks):
        feats = order[start:start + fill]
        for d in range(DEG):
            for g in np.unique(blocks[d, feats]):
                g = int(g)
                rows = np.nonzero(blocks[d, feats] == g)[0]
                Z = np.zeros((CHUNK, 128), np.float16)
                Z[rows, idx[d, feats[rows]] - 128 * g] = sgn[d, feats[rows]]
                items[d][g].append((ci, Z))
    for d in range(DEG):
        for g in range(NBLK):
            if not items[d][g]:
                items[d][g].append((0, np.zeros((CHUNK, 128), np.float16)))

    # pair Z mats into 256-col rows (512B DMA lines in fp16)
    zmats = []
    plan = [[[] for _ in range(NBLK)] for _ in range(DEG)]
    for d in range(DEG):
        for g in range(NBLK):
            lst = sorted(items[d][g], key=lambda x: x[0])
            for i, (ci, Z) in enumerate(lst):
                plan[d][g].append((ci, len(zmats)))
                zmats.append(Z)
            if len(lst) % 2:
                zmats.append(np.zeros((CHUNK, 128), np.float16))  # pad pair
    npair = len(zmats) // 2
    zm = np.stack(zmats).reshape(npair, 2, CHUNK, 128)  # [P, 2, 128, 128]
    zm_t = np.ascontiguousarray(
        zm.transpose(2, 0, 1, 3).reshape(CHUNK, npair, 256))
    return order, chunks, plan, zm_t


def build_dft_tables():
    # stage-2 weights [128, 4, 512] fp16: for partition p, block q the
    # contraction row is n = 128 q + p. Column quarters:
    #   [0:128)   ReA: k = 0..127      cos(2 pi n k / 512) * S2SCALE
    #   [128:256) ReB: k = 128..255
    #   [256:384) ImA: k = 0..127     -sin(...) * S2SCALE, except col 256
    #             (k=0, identically zero) carries the Nyquist row cos(pi n)
    #   [384:512) ImB: k = 128..255
    n = (128 * np.arange(4)[:, None, None] + np.arange(128)[None, :, None]
         ).astype(np.float64)                      # [4, 128, 1]
    k = np.arange(128)[None, None, :]              # [1, 1, 128]
    angA = 2 * np.pi * n * k / NCOMP
    angB = 2 * np.pi * n * (k + 128) / NCOMP
    reA = np.cos(angA)
    reB = np.cos(angB)
    imA = -np.sin(angA)
    imB = -np.sin(angB)
    imA[:, :, 0] = np.cos(np.pi * n[:, :, 0])      # Nyquist in the Im(0) slot
    dft = np.concatenate([reA, reB, imA, imB], axis=2) * S2SCALE  # [4,128,512]
    dft_t = np.ascontiguousarray(dft.transpose(1, 0, 2)).astype(np.float16)

    # stage-4 weights [128, 4, 512] fp16: prod quarter q, partition p maps to
    # spectrum bin k (q0: k=p, q1: k=128+p, q2: im k=p with p=0 the Nyquist
    # product, q3: im k=128+p). Output col c = n. Scale 16^3 / NCOMP = 8.
    SC = (1.0 / S2SCALE) ** 3 / NCOMP
    nn = np.arange(NCOMP)[None, :]
    p = np.arange(128)[:, None]
    ico = np.zeros((4, 128, NCOMP), np.float64)
    ck = np.where(p == 0, 1.0, 2.0)
    ico[0] = ck * np.cos(2 * np.pi * p * nn / NCOMP) * SC
    ico[1] = 2.0 * np.cos(2 * np.pi * (p + 128) * nn / NCOMP) * SC
    ico[2] = -2.0 * np.sin(2 * np.pi * p * nn / NCOMP) * SC
    ico[2, 0] = np.cos(np.pi * nn[0]) * SC         # Nyquist column
    ico[3] = -2.0 * np.sin(2 * np.pi * (p + 128) * nn / NCOMP) * SC
    ico_t = np.ascontiguousarray(ico.transpose(1, 0, 2)).astype(np.float16)
    return dft_t, ico_t


def build_program(plan, chunks, npair):
    nch = len(chunks)
    ngrp = (nch + GRP - 1) // GRP
    fills = [f for (_, f) in chunks]
    # (d, g) -> pair-range [lo, hi) into the z stack, and item slots
    zoff = {}
    pos = 0
    for d in range(DEG):
        for g in range(NBLK):
            n = len(plan[d][g])
            zoff[(d, g)] = pos
            pos += (n + 1) // 2
    assert pos == npair

    nc = bacc.Bacc("TRN2", target_bir_lowering=False, debug=False)
    xp = nc.dram_tensor("xp", [nch * 128, B_CORE], F16,
                        kind="ExternalInput").ap()
    zm = nc.dram_tensor("zm", [128, npair, 256], F16,
                        kind="ExternalInput").ap()
    dft = nc.dram_tensor("dft", [128, 4, 512], F16, kind="ExternalInput").ap()
    ico = nc.dram_tensor("ico", [128, 4, 512], F16, kind="ExternalInput").ap()
    ot = nc.dram_tensor("ot", [NCOMP, B_CORE], F32, kind="ExternalOutput").ap()

    with tile.TileContext(nc) as tc:
        with (
            tc.tile_pool(name="pz", bufs=1) as pz,
            tc.tile_pool(name="pc", bufs=1) as pc,
            tc.tile_pool(name="px", bufs=1) as px,
            tc.tile_pool(name="psk", bufs=1) as psk,
            tc.tile_pool(name="pfr", bufs=1) as pfr,
            tc.tile_pool(name="pprod", bufs=2) as pprod,
            tc.tile_pool(name="ptmp", bufs=1) as ptmp,
            tc.tile_pool(name="pout", bufs=4) as pout,
            tc.tile_pool(name="ps_sk", bufs=2, space="PSUM") as ps_sk,
            tc.tile_pool(name="ps_fp", bufs=3, space="PSUM") as ps_fp,
        ):
            zt = pz.tile([128, npair, 256], F16, tag="zt")
            xgs = {}

            def load_z(d, g0, g1):
                # one DMA covering (d, g0..g1)'s pair range
                lo = zoff[(d, g0)]
                hi = zoff[(d, g1)] + (len(plan[d][g1]) + 1) // 2
                nc.sync.dma_start(zt[:, lo:hi, :], zm[:, lo:hi, :])

            def load_xg(t, j):
                w = min(GRP, nch - GRP * j)
                xt = px.tile([128, w, BT], F16, tag=f"xg{t}_{j}")
                src = xp[128 * GRP * j:128 * (GRP * j + w),
                         BT * t:BT * (t + 1)]
                nc.sync.dma_start(xt[:], src.rearrange("(c p) n -> p c n",
                                                       p=128))
                xgs[(t, j)] = xt

            def zsl(slot, fill):
                return zt[0:fill, slot // 2, 128 * (slot % 2):
                          128 * (slot % 2) + 128]

            # ---- DMA order: interleave z/X for btile-0 arrival, then rest
            load_z(0, 0, 1)
            load_xg(0, 0)
            load_z(0, 2, 3)
            load_xg(0, 1)
            load_xg(0, 2)
            load_z(1, 0, 3)
            load_xg(0, 3)
            load_xg(0, 4)
            load_z(2, 0, 3)
            for j in range(5, ngrp):
                load_xg(0, j)
            dftt = pc.tile([128, 4, 512], F16, tag="dftt")
            nc.gpsimd.dma_start(dftt[:], dft[:])
            icot = pc.tile([128, 4, 512], F16, tag="icot")
            nc.gpsimd.dma_start(icot[:], ico[:])
            for j in range(ngrp):
                load_xg(1, j)

            prods = {}

            def emit_stage4(t):
                prod = prods.pop(t)
                for m in range(4):
                    po = ps_fp.tile([128, BT], F32, tag="fp")
                    for q in range(4):
                        nc.tensor.matmul(
                            po[:], icot[:, q, 128 * m:128 * (m + 1)],
                            prod[:, q, :],
                            start=(q == 0), stop=(q == 3))
                    ob = pout.tile([128, BT], F32, tag="ob")
                    nc.scalar.copy(ob[:], po[:])
                    nc.gpsimd.dma_start(
                        ot[128 * m:128 * (m + 1), BT * t:BT * (t + 1)], ob[:])

            def emit_stage3_half(frs, prod, h):
                # complex product over degrees on DVE, pure-SBUF fp16 (2x)
                f0, f1, f2 = frs
                re, im = h, 2 + h
                t1 = ptmp.tile([128, BT], F16, tag="t1")
                t2 = ptmp.tile([128, BT], F16, tag="t2")
                pre01 = ptmp.tile([128, BT], F16, tag="t3")
                pim01 = ptmp.tile([128, BT], F16, tag="t4")
                nc.vector.tensor_mul(t1[:], f0[:, re, :], f1[:, re, :])
                nc.vector.tensor_mul(t2[:], f0[:, im, :], f1[:, im, :])
                nc.vector.tensor_sub(pre01[:], t1[:], t2[:])
                nc.vector.tensor_mul(t1[:], f0[:, re, :], f1[:, im, :])
                nc.vector.tensor_mul(t2[:], f0[:, im, :], f1[:, re, :])
                nc.vector.tensor_add(pim01[:], t1[:], t2[:])
                nc.vector.tensor_mul(t1[:], pre01[:], f2[:, re, :])
                nc.vector.tensor_mul(t2[:], pim01[:], f2[:, im, :])
                nc.vector.tensor_sub(prod[:, re, :], t1[:], t2[:])
                nc.vector.tensor_mul(t1[:], pre01[:], f2[:, im, :])
                nc.vector.tensor_mul(t2[:], pim01[:], f2[:, re, :])
                nc.vector.tensor_add(prod[:, im, :], t1[:], t2[:])
                if h == 0:
                    # row-0 fixups: DC (quarter 0) and Nyquist (quarter 2)
                    # are real products, clobbered by the complex-mul mixing
                    for qq in (0, 2):
                        tr = ptmp.tile([1, BT], F16, tag=f"r{qq}")
                        nc.vector.tensor_mul(tr[:], f0[0:1, qq, :],
                                             f1[0:1, qq, :])
                        nc.vector.tensor_mul(prod[0:1, qq, :], tr[:],
                                             f2[0:1, qq, :])

            for t in range(NBT):
                frs = []
                skds = []
                for d in range(DEG):
                    # ---- stage 1 (count sketch) for degree d
                    skd = psk.tile([128, 4, BT], F16, tag=f"sk{d}")
                    skds.append(skd)
                    for g in range(NBLK):
                        pssk = ps_sk.tile([128, BT], F32, tag="psk")
                        items = plan[d][g]
                        for i, (ci, slot) in enumerate(items):
                            fill = fills[ci]
                            nc.tensor.matmul(
                                pssk[:],
                                zsl(slot, fill),
                                xgs[(t, ci // GRP)][0:fill, ci % GRP, :],
                                start=(i == 0),
                                stop=(i == len(items) - 1),
                            )
                        nc.scalar.copy(skd[:, g, :], pssk[:])
                    frd = pfr.tile([128, 4, BT], F16, tag=f"fr{d}")
                    frs.append(frd)

                # ---- stage 2 (rfft), quarter-major so stage-3 half A can
                # start while half B's quarters are still on the PE
                prod = pprod.tile([128, 4, BT], F16, tag="prod")
                for qq in (0, 2, 1, 3):
                    for d in range(DEG):
                        psfr = ps_fp.tile([128, BT], F32, tag="fp")
                        for q in range(4):
                            nc.tensor.matmul(
                                psfr[:], dftt[:, q, 128 * qq:128 * (qq + 1)],
                                skds[d][:, q, :],
                                start=(q == 0), stop=(q == 3))
                        nc.scalar.copy(frs[d][:, qq, :], psfr[:])
                    if qq == 2:
                        emit_stage3_half(frs, prod, 0)
                # ---- stage 4 of the PREVIOUS btile overlaps stage 3
                if t > 0:
                    emit_stage4(t - 1)
                emit_stage3_half(frs, prod, 1)
                prods[t] = prod
            emit_stage4(NBT - 1)

    nc.compile()
    return nc


def prepare_inputs(X, index_hash, bit_hash):
    order, chunks, plan, zm_t = build_plan(index_hash, bit_hash)
    dft_t, ico_t = build_dft_tables()
    npair = zm_t.shape[1]
    # padded layout: chunk c occupies rows [128c, 128c+fill), rest zero
    Xt = np.asarray(X, np.float32).T[order].astype(np.float16)
    Xp = np.zeros((len(chunks) * 128, Xt.shape[1]), np.float16)
    for c, (start, fill) in enumerate(chunks):
        Xp[128 * c:128 * c + fill] = Xt[start:start + fill]
    shared = {"zm": zm_t, "dft": dft_t, "ico": ico_t}
    return plan, chunks, npair, Xp, shared


def kernel(X, index_hash, bit_hash):
    plan, chunks, npair, Xp, shared = prepare_inputs(X, index_hash, bit_hash)
    nc = build_program(plan, chunks, npair)
    in_maps = [
        {"xp": np.ascontiguousarray(Xp[:, c * B_CORE:(c + 1) * B_CORE]),
         **shared}
        for c in range(NCORES)
    ]
    res = bass_utils.run_bass_kernel_spmd(
        nc, in_maps, core_ids=list(range(NCORES)))
    out = np.empty((B, NCOMP), np.float32)
    for c in range(NCORES):
        out[c * B_CORE:(c + 1) * B_CORE] = res.results[c]["ot"].T
    return out


# revision 9
# speedup vs baseline: 1.1934x; 1.0182x over previous
"""PolyCntSketch (TensorSketch, degree 3) Trainium2 kernel — v2 (fp16).

Math: for each degree d, CountSketch_d = X @ S_d (S_d one-hot signed), then
out = irfft(prod_d rfft(CountSketch_d)).

Device strategy (pure data parallelism over batch, 8 cores):
  - Host feeds X transposed ([F, B_core]) in fp16, features packed into
    128-row chunks holding whole (block_d0, block_d1, block_d2)-classes
    (block = idx_d // 128), so each chunk touches few 128-bucket blocks per
    degree -> few segment matmuls.
  - Stage 1: count sketch via per-(chunk, degree, block) fp16 matmuls
    (lhsT = signed one-hot Z) accumulated in PSUM -> sketch [512, B].
  - Stage 2: rfft as fp16 DFT matmul, scaled by 1/16 to keep fp16 range.
    The Nyquist bin's coefficients ride in the identically-zero Im(0)
    weight column, so no extra sum/matmul is needed.
  - Stage 3: complex product across the 3 degrees on DVE in pure-SBUF fp16
    (2x mode); row 0 (DC & Nyquist, both real) fixed up with tiny ops.
  - Stage 4: irfft as fp16 matmul (table scaled by 16^3) -> out^T fp32.
Two batch tiles of 512 columns (one PSUM bank per matmul output).
"""
import sys

for _p in ("/opt/trn_rl_repo",):
    if _p not in sys.path:
        sys.path.append(_p)

import numpy as np

from concourse import bacc, mybir, tile
from concourse import bass_utils

F16 = mybir.dt.float16
F32 = mybir.dt.float32

B, F, NCOMP, DEG = 8192, 4096, 512, 3
NCORES = 8
B_CORE = B // NCORES
BT = 512                     # batch columns per tile
NBT = B_CORE // BT
CHUNK = 128
NBLK = NCOMP // 128          # 4 bucket blocks
GRP = 4                      # chunks per X-load group
S2SCALE = 1.0 / 16.0         # fp16 range scaling for the DFT stage


def build_plan(index_hash, bit_hash):
    """FFD-pack whole (g0,g1,g2)-classes into 128-row chunks.

    Returns:
      order [F]: feature order for the transposed X upload
      chunks: list of (start, fill) row ranges into the ordered X
      plan[d][g]: list of (chunk_idx, zslot) in emission order ((d,g)-major)
      zm_t [128, npair, 256]: stacked Z matrices, two per 256-col row
    """
    idx = np.asarray(index_hash)
    sgn = (np.asarray(bit_hash) * 2 - 1).astype(np.float32)
    blocks = idx >> 7
    key = blocks[0] * 16 + blocks[1] * 4 + blocks[2]
    order_all = np.argsort(key, kind="stable")
    kvals = key[order_all]

    from collections import defaultdict
    sgroups = defaultdict(list)
    for kv in np.unique(kvals):
        f = order_all[kvals == kv]
        kv = int(kv)
        while len(f) > CHUNK:
            sgroups[kv >> 2].append((kv, f[:CHUNK]))
            f = f[CHUNK:]
        if len(f):
            sgroups[kv >> 2].append((kv, f))

    bins = []
    for sg in sorted(sgroups):
        sbins = []
        for kv, f in sorted(sgroups[sg], key=lambda x: -len(x[1])):
            for b in sbins:
                if sum(len(x[1]) for x in b) + len(f) <= CHUNK:
                    b.append((kv, f))
                    break
            else:
                sbins.append([(kv, f)])
        bins.extend(sbins)
    # merge small bins globally (saves chunks at the cost of 1-2 extra mms)
    bins.sort(key=lambda b: sum(len(x[1]) for x in b))
    merged = []
    while bins:
        b = bins.pop(0)
        size = sum(len(x[1]) for x in b)
        for i in range(len(bins) - 1, -1, -1):
            if sum(len(x[1]) for x in bins[i]) + size <= CHUNK:
                bins[i].extend(b)
                break
        else:
            merged.append(b)
    merged.sort(key=lambda b: min(x[0] for x in b))

    order = []
    chunks = []
    for b in merged:
        start = len(order)
        for kv, f in b:
            order.extend(f.tolist())
        chunks.append((start, len(order) - start))
    order = np.array(order)
    assert len(order) == F and len(np.unique(order)) == F

    items = [[[] for _ in range(NBLK)] for _ in range(DEG)]
    for ci, (start, fill) in enumerate(chunks):
        feats = order[start:start + fill]
        for d in range(DEG):
            for g in np.unique(blocks[d, feats]):
                g = int(g)
                rows = np.nonzero(blocks[d, feats] == g)[0]
                Z = np.zeros((CHUNK, 128), np.float16)
                Z[rows, idx[d, feats[rows]] - 128 * g] = sgn[d, feats[rows]]
                items[d][g].append((ci, Z))
    for d in range(DEG):
        for g in range(NBLK):
            if not items[d][g]:
                items[d][g].append((0, np.zeros((CHUNK, 128), np.float16)))

    # pair Z mats into 256-col rows (512B DMA lines in fp16)
    zmats = []
    plan = [[[] for _ in range(NBLK)] for _ in range(DEG)]
    for d in range(DEG):
        for g in range(NBLK):
            lst = sorted(items[d][g], key=lambda x: x[0])
            for i, (ci, Z) in enumerate(lst):
                plan[d][g].append((ci, len(zmats)))
                zmats.append(Z)
            if len(lst) % 2:
                zmats.append(np.zeros((CHUNK, 128), np.float16))  # pad pair
    npair = len(zmats) // 2
    zm = np.stack(zmats).reshape(npair, 2, CHUNK, 128)  # [P, 2, 128, 128]
    zm_t = np.ascontiguousarray(
        zm.transpose(2, 0, 1, 3).reshape(CHUNK, npair, 256))
    return order, chunks, plan, zm_t


def build_dft_tables():
    # stage-2 weights [128, 4, 512] fp16: for partition p, block q the
    # contraction row is n = 128 q + p. Column quarters:
    #   [0:128)   ReA: k = 0..127      cos(2 pi n k / 512) * S2SCALE
    #   [128:256) ReB: k = 128..255
    #   [256:384) ImA: k = 0..127     -sin(...) * S2SCALE, except col 256
    #             (k=0, identically zero) carries the Nyquist row cos(pi n)
    #   [384:512) ImB: k = 128..255
    n = (128 * np.arange(4)[:, None, None] + np.arange(128)[None, :, None]
         ).astype(np.float64)                      # [4, 128, 1]
    k = np.arange(128)[None, None, :]              # [1, 1, 128]
    angA = 2 * np.pi * n * k / NCOMP
    angB = 2 * np.pi * n * (k + 128) / NCOMP
    reA = np.cos(angA)
    reB = np.cos(angB)
    imA = -np.sin(angA)
    imB = -np.sin(angB)
    imA[:, :, 0] = np.cos(np.pi * n[:, :, 0])      # Nyquist in the Im(0) slot
    dft = np.concatenate([reA, reB, imA, imB], axis=2) * S2SCALE  # [4,128,512]
    dft_t = np.ascontiguousarray(dft.transpose(1, 0, 2)).astype(np.float16)

    # stage-4 weights [128, 4, 512] fp16: prod quarter q, partition p maps to
    # spectrum bin k (q0: k=p, q1: k=128+p, q2: im k=p with p=0 the Nyquist
    # product, q3: im k=128+p). Output col c = n. Scale 16^3 / NCOMP = 8.
    SC = (1.0 / S2SCALE) ** 3 / NCOMP
    nn = np.arange(NCOMP)[None, :]
    p = np.arange(128)[:, None]
    ico = np.zeros((4, 128, NCOMP), np.float64)
    ck = np.where(p == 0, 1.0, 2.0)
    ico[0] = ck * np.cos(2 * np.pi * p * nn / NCOMP) * SC
    ico[1] = 2.0 * np.cos(2 * np.pi * (p + 128) * nn / NCOMP) * SC
    ico[2] = -2.0 * np.sin(2 * np.pi * p * nn / NCOMP) * SC
    ico[2, 0] = np.cos(np.pi * nn[0]) * SC         # Nyquist column
    ico[3] = -2.0 * np.sin(2 * np.pi * (p + 128) * nn / NCOMP) * SC
    ico_t = np.ascontiguousarray(ico.transpose(1, 0, 2)).astype(np.float16)
    return dft_t, ico_t


def build_program(plan, chunks, npair):
    nch = len(chunks)
    ngrp = (nch + GRP - 1) // GRP
    fills = [f for (_, f) in chunks]
    # (d, g) -> pair-range [lo, hi) into the z stack, and item slots
    zoff = {}
    pos = 0
    for d in range(DEG):
        for g in range(NBLK):
            n = len(plan[d][g])
            zoff[(d, g)] = pos
            pos += (n + 1) // 2
    assert pos == npair

    nc = bacc.Bacc("TRN2", target_bir_lowering=False, debug=False)
    xp = nc.dram_tensor("xp", [nch * 128, B_CORE], F16,
                        kind="ExternalInput").ap()
    zm = nc.dram_tensor("zm", [128, npair, 256], F16,
                        kind="ExternalInput").ap()
    dft = nc.dram_tensor("dft", [128, 4, 512], F16, kind="ExternalInput").ap()
    ico = nc.dram_tensor("ico", [128, 4, 512], F16, kind="ExternalInput").ap()
    ot = nc.dram_tensor("ot", [NCOMP, B_CORE], F32, kind="ExternalOutput").ap()

    with tile.TileContext(nc) as tc:
        with (
            tc.tile_pool(name="pz", bufs=1) as pz,
            tc.tile_pool(name="pc", bufs=1) as pc,
            tc.tile_pool(name="px", bufs=1) as px,
            tc.tile_pool(name="psk", bufs=1) as psk,
            tc.tile_pool(name="pfr", bufs=1) as pfr,
            tc.tile_pool(name="pprod", bufs=2) as pprod,
            tc.tile_pool(name="ptmp", bufs=1) as ptmp,
            tc.tile_pool(name="pout", bufs=4) as pout,
            tc.tile_pool(name="ps_ska", bufs=1, space="PSUM") as ps_ska,
            tc.tile_pool(name="ps_sk", bufs=2, space="PSUM") as ps_sk,
            tc.tile_pool(name="ps_fp", bufs=2, space="PSUM") as ps_fp,
        ):
            zt = pz.tile([128, npair, 256], F16, tag="zt")
            xgs = {}

            def load_z(d, g0, g1):
                # one DMA covering (d, g0..g1)'s pair range
                lo = zoff[(d, g0)]
                hi = zoff[(d, g1)] + (len(plan[d][g1]) + 1) // 2
                nc.sync.dma_start(zt[:, lo:hi, :], zm[:, lo:hi, :])

            def load_xg(t, j):
                w = min(GRP, nch - GRP * j)
                xt = px.tile([128, w, BT], F16, tag=f"xg{t}_{j}")
                src = xp[128 * GRP * j:128 * (GRP * j + w),
                         BT * t:BT * (t + 1)]
                nc.sync.dma_start(xt[:], src.rearrange("(c p) n -> p c n",
                                                       p=128))
                xgs[(t, j)] = xt

            def zsl(slot, fill):
                return zt[0:fill, slot // 2, 128 * (slot % 2):
                          128 * (slot % 2) + 128]

            # ---- DMA order. sync queue: z0, X(t0) with z1 slotted in, X(t1).
            # gpsimd queue (concurrent): z2, dft, ico — arrives while the PE
            # chews through arrival-ordered d0/d1 work.
            load_z(0, 0, 3)
            load_xg(0, 0)
            load_xg(0, 1)
            load_xg(0, 2)
            load_z(1, 0, 3)
            for j in range(3, ngrp):
                load_xg(0, j)
            load_z2 = lambda: nc.gpsimd.dma_start(
                zt[:, zoff[(2, 0)]:npair, :], zm[:, zoff[(2, 0)]:npair, :])
            load_z2()
            dftt = pc.tile([128, 4, 512], F16, tag="dftt")
            nc.gpsimd.dma_start(dftt[:], dft[:])
            icot = pc.tile([128, 4, 512], F16, tag="icot")
            nc.gpsimd.dma_start(icot[:], ico[:])
            for j in range(ngrp):
                load_xg(1, j)

            prods = {}

            def emit_stage4(t):
                prod = prods.pop(t)
                for m in range(4):
                    po = ps_fp.tile([128, BT], F32, tag="fp")
                    for q in range(4):
                        nc.tensor.matmul(
                            po[:], icot[:, q, 128 * m:128 * (m + 1)],
                            prod[:, q, :],
                            start=(q == 0), stop=(q == 3))
                    ob = pout.tile([128, BT], F32, tag="ob")
                    nc.scalar.copy(ob[:], po[:])
                    nc.gpsimd.dma_start(
                        ot[128 * m:128 * (m + 1), BT * t:BT * (t + 1)], ob[:])

            def emit_stage3_half(frs, prod, h):
                # complex product over degrees on DVE, pure-SBUF fp16 (2x)
                f0, f1, f2 = frs
                re, im = h, 2 + h
                t1 = ptmp.tile([128, BT], F16, tag="t1")
                t2 = ptmp.tile([128, BT], F16, tag="t2")
                pre01 = ptmp.tile([128, BT], F16, tag="t3")
                pim01 = ptmp.tile([128, BT], F16, tag="t4")
                nc.vector.tensor_mul(t1[:], f0[:, re, :], f1[:, re, :])
                nc.vector.tensor_mul(t2[:], f0[:, im, :], f1[:, im, :])
                nc.vector.tensor_sub(pre01[:], t1[:], t2[:])
                nc.vector.tensor_mul(t1[:], f0[:, re, :], f1[:, im, :])
                nc.vector.tensor_mul(t2[:], f0[:, im, :], f1[:, re, :])
                nc.vector.tensor_add(pim01[:], t1[:], t2[:])
                nc.vector.tensor_mul(t1[:], pre01[:], f2[:, re, :])
                nc.vector.tensor_mul(t2[:], pim01[:], f2[:, im, :])
                nc.vector.tensor_sub(prod[:, re, :], t1[:], t2[:])
                nc.vector.tensor_mul(t1[:], pre01[:], f2[:, im, :])
                nc.vector.tensor_mul(t2[:], pim01[:], f2[:, re, :])
                nc.vector.tensor_add(prod[:, im, :], t1[:], t2[:])
                if h == 0:
                    # row-0 fixups: DC (quarter 0) and Nyquist (quarter 2)
                    # are real products, clobbered by the complex-mul mixing
                    for qq in (0, 2):
                        tr = ptmp.tile([1, BT], F16, tag=f"r{qq}")
                        nc.vector.tensor_mul(tr[:], f0[0:1, qq, :],
                                             f1[0:1, qq, :])
                        nc.vector.tensor_mul(prod[0:1, qq, :], tr[:],
                                             f2[0:1, qq, :])

            # per-chunk item lists for arrival-ordered emission on btile 0
            by_chunk = [[[] for _ in range(nch)] for _ in range(2)]
            for d in (0, 1):
                for g in range(NBLK):
                    items = plan[d][g]
                    for i, (ci, slot) in enumerate(items):
                        by_chunk[d][ci].append(
                            (g, slot, i == 0, i == len(items) - 1))

            def s1mm(ps, t, ci, slot, st, sp):
                fill = fills[ci]
                nc.tensor.matmul(
                    ps, zsl(slot, fill),
                    xgs[(t, ci // GRP)][0:fill, ci % GRP, :],
                    start=st, stop=sp)

            def emit_s1_dgmajor(t, d, skd):
                for g in range(NBLK):
                    pssk = ps_sk.tile([128, BT], F32, tag="psk")
                    items = plan[d][g]
                    for i, (ci, slot) in enumerate(items):
                        s1mm(pssk[:], t, ci, slot, i == 0,
                             i == len(items) - 1)
                    nc.scalar.copy(skd[:, g, :], pssk[:])

            CATCH = 16   # chunk index by which z1 should have landed

            for t in range(NBT):
                frs = []
                skds = []
                for d in range(DEG):
                    skd = psk.tile([128, 4, BT], F16, tag=f"sk{d}")
                    skds.append(skd)
                    frd = pfr.tile([128, 4, BT], F16, tag=f"fr{d}")
                    frs.append(frd)

                if t == 0:
                    # ---- arrival-ordered stage 1: d0 into one bank per
                    # block (safe PE-write/ACT-read separation), d1 groups
                    # 0-1 interleaved once z1 is in, 2-3 caught up after,
                    # d2 (d,g)-major once z2 is in. Keeps the PE dense while
                    # X(t0) streams in.
                    ska = ps_ska.tile([128, 4, BT], F32, tag="ska")
                    d1ps = {}

                    def emit_d1(ci, gset):
                        for (g, slot, st, sp) in by_chunk[1][ci]:
                            if g not in gset:
                                continue
                            if st:
                                pd = ps_sk.tile([128, BT], F32, tag="psk")
                                d1ps[g] = pd
                            s1mm(d1ps[g][:], 0, ci, slot, st, sp)
                            if sp:
                                nc.scalar.copy(skds[1][:, g, :], d1ps[g][:])

                    for ci in range(nch):
                        for (g, slot, st, sp) in by_chunk[0][ci]:
                            s1mm(ska[:, g, :], 0, ci, slot, st, sp)
                            if sp:
                                nc.scalar.copy(skds[0][:, g, :], ska[:, g, :])
                        if ci == CATCH:
                            for cj in range(CATCH):
                                emit_d1(cj, (0, 1))
                        if ci >= CATCH:
                            emit_d1(ci, (0, 1))
                    for ci in range(nch):
                        emit_d1(ci, (2, 3))
                    emit_s1_dgmajor(0, 2, skds[2])
                else:
                    for d in range(DEG):
                        emit_s1_dgmajor(t, d, skds[d])

                # ---- stage 2 (rfft), quarter-major so stage-3 half A can
                # start while half B's quarters are still on the PE
                prod = pprod.tile([128, 4, BT], F16, tag="prod")
                for qq in (0, 2, 1, 3):
                    for d in range(DEG):
                        psfr = ps_fp.tile([128, BT], F32, tag="fp")
                        for q in range(4):
                            nc.tensor.matmul(
                                psfr[:], dftt[:, q, 128 * qq:128 * (qq + 1)],
                                skds[d][:, q, :],
                                start=(q == 0), stop=(q == 3))
                        nc.scalar.copy(frs[d][:, qq, :], psfr[:])
                    if qq == 2:
                        emit_stage3_half(frs, prod, 0)
                # ---- stage 4 of the PREVIOUS btile overlaps stage 3
                if t > 0:
                    emit_stage4(t - 1)
                emit_stage3_half(frs, prod, 1)
                prods[t] = prod
            emit_stage4(NBT - 1)

    nc.compile()
    return nc


def prepare_inputs(X, index_hash, bit_hash):
    order, chunks, plan, zm_t = build_plan(index_hash, bit_hash)
    dft_t, ico_t = build_dft_tables()
    npair = zm_t.shape[1]
    # padded layout: chunk c occupies rows [128c, 128c+fill), rest zero
    Xt = np.asarray(X, np.float32).T[order].astype(np.float16)
    Xp = np.zeros((len(chunks) * 128, Xt.shape[1]), np.float16)
    for c, (start, fill) in enumerate(chunks):
        Xp[128 * c:128 * c + fill] = Xt[start:start + fill]
    shared = {"zm": zm_t, "dft": dft_t, "ico": ico_t}
    return plan, chunks, npair, Xp, shared


def kernel(X, index_hash, bit_hash):
    plan, chunks, npair, Xp, shared = prepare_inputs(X, index_hash, bit_hash)
    nc = build_program(plan, chunks, npair)
    in_maps = [
        {"xp": np.ascontiguousarray(Xp[:, c * B_CORE:(c + 1) * B_CORE]),
         **shared}
        for c in range(NCORES)
    ]
    res = bass_utils.run_bass_kernel_spmd(
        nc, in_maps, core_ids=list(range(NCORES)))
    out = np.empty((B, NCOMP), np.float32)
    for c in range(NCORES):
        out[c * B_CORE:(c + 1) * B_CORE] = res.results[c]["ot"].T
    return out
